# revision 42
# baseline (speedup 1.0000x reference)
"""NodeNet GNN message-passing kernel for 8 Trainium2 NeuronCores.

Strategy (per sharding hint): shard nodes across the 8 cores; partition
edges by destination node on the host so the scatter-mean is device-local.

Per core (12,500 real nodes, padded to 12,544 = 196 windows of 64 nodes):
  - Host sorts edges by destination and pre-scales each edge row by
    1/count(dst), so the device segment-sum directly yields the mean.
    Each 64-node window's edge list is padded to a multiple of 128; each
    core processes its windows in descending-edge-count order so the
    shared (SPMD) per-window chunk counts CB[j] = max-over-cores of the
    j-th order statistic waste minimal padding, and the smallest windows
    land at the end, shortening the pipeline drain.  Edge features are
    laid out chunk-transposed fp16 ([128, nch*128]), with each MLP
    group's node features interleaved into the same stream, so the whole
    input arrives as one wide contiguous DMA per group.
  - Device builds, per 128-edge chunk, a [128 edge, 64 node] fp16
    one-hot (is_equal of dst-rel against an iota ramp, VectorE) and
    contracts it on the TensorEngine:
    meanT[d, n] += matmul(lhsT=attr[e, d], rhs=onehot[e, n]) accumulated
    in PSUM (fp32).  Everything stays feature-major so the 3-layer MLP
    (fp16 matmuls, fp32 PSUM accumulate, ScalarE relu+bias evacuations)
    chains with no transposes: h1T = relu(W1.T @ [xT; meanT] + b1), ...
  - Output is accumulated feature-major fp16 in SBUF and stored with one
    deferred DMA per group; the host transposes, upcasts, and un-permutes.

Cost-model timeline (per core): ~191 us, ~90% of the 173 us DMA-byte
floor (~62 MB/core at ~360 GB/s); VectorE/ScalarE/TensorE all at or
below ~65% occupancy, fully hidden behind the edge-feature stream.
"""

import numpy as np

import concourse.bacc as bacc
import concourse.mybir as mybir
import concourse.tile as tile
from concourse.bass_utils import run_bass_kernel_spmd

P = 128                    # partitions / matmul contraction tile
D = 128                    # node & edge feature dim
HIDDEN = 256
DOUT = 128
N_NODES = 100000
N_CORES = 8
NPC_REAL = 12500           # real nodes per core
W = 64                     # nodes per binning window
WINDOWS = 196              # windows per core (196*64 = 12544)
NPC = WINDOWS * W          # padded nodes per core
GPW = 8                    # windows per MLP group (512 nodes)
GROUP_N = GPW * W
ATTR_BUFS = 3
OH_BUFS = 6
ACT_BUFS = 4
PBIN_BUFS = 3

_prog_cache: dict = {}

f32 = mybir.dt.float32
f16 = mybir.dt.float16


def _group_sizes():
    gsizes = []
    rem = WINDOWS
    while rem > 2 * GPW:
        gsizes.append(GPW)
        rem -= GPW
    for t in (GPW // 2, GPW // 2, GPW // 4, GPW // 4):
        t = min(t, rem)
        if t > 0:
            gsizes.append(t)
            rem -= t
    while rem > 0:
        t = min(GPW // 4, rem)
        gsizes.append(t)
        rem -= t
    return gsizes


def _build_program(CB, ablate=()):
    """Build the Bass/Tile program. CB[j] = number of 128-edge chunks for
    window j (identical across cores; per-core data is padded to match).
    ablate: subset of {"mlp", "bin", "oh"} to skip stages (sim studies)."""
    CB = list(CB)
    CBmax = max(CB)
    offs = np.concatenate([[0], np.cumsum(CB)]).astype(int)
    NCH = int(offs[-1])

    nc = bacc.Bacc(None)
    # attrT carries, per group: the edge-feature chunks, then the group's
    # node features (gsz*W fp16 columns) — one combined DMA per group.
    attrT_d = nc.dram_tensor(
        "attrT", [P, NCH * D + WINDOWS * W], f16, kind="ExternalInput"
    )
    # fp16 consts: iota ramp (CBmax*W) | dstrel (NCH)
    c16_d = nc.dram_tensor("c16", [P, CBmax * W + NCH], f16, kind="ExternalInput")
    # fp32 consts: 5 bias columns
    consts_d = nc.dram_tensor("consts", [P, 5], f32, kind="ExternalInput")
    wts_d = nc.dram_tensor("wts", [P, 4 * HIDDEN + 2 * DOUT], f16,
                           kind="ExternalInput")
    outT_d = nc.dram_tensor("outT", [P, NPC], f16, kind="ExternalOutput")

    Relu = mybir.ActivationFunctionType.Relu
    Ident = mybir.ActivationFunctionType.Identity

    with tile.TileContext(nc) as tc:
        with (
            tc.tile_pool(name="const", bufs=1) as cpool,
            tc.tile_pool(name="attr", bufs=ATTR_BUFS) as apool,
            tc.tile_pool(name="oh", bufs=OH_BUFS) as ohpool,
            tc.tile_pool(name="acts", bufs=ACT_BUFS) as actpool,
            tc.tile_pool(name="pbin", bufs=PBIN_BUFS, space="PSUM") as pbin,
            tc.tile_pool(name="pmlp", bufs=1, space="PSUM") as pmlp,
        ):
            # --- constants (tiles now; DMAs after the first attr DMA so
            # the edge stream starts immediately) ---
            cs = cpool.tile([P, 5], f32, tag="consts")
            ws = cpool.tile([P, 4 * HIDDEN + 2 * DOUT], f16, tag="wts")
            c16 = cpool.tile([P, CBmax * W + NCH], f16, tag="c16")
            w1s_0 = ws[:, 0:HIDDEN]
            w1s_1 = ws[:, HIDDEN : 2 * HIDDEN]
            w2s_0 = ws[:, 2 * HIDDEN : 3 * HIDDEN]
            w2s_1 = ws[:, 3 * HIDDEN : 4 * HIDDEN]
            w3s_0 = ws[:, 4 * HIDDEN : 4 * HIDDEN + DOUT]
            w3s_1 = ws[:, 4 * HIDDEN + DOUT : 4 * HIDDEN + 2 * DOUT]
            b1s_0 = cs[:, 0:1]
            b1s_1 = cs[:, 1:2]
            b2s_0 = cs[:, 2:3]
            b2s_1 = cs[:, 3:4]
            b3s = cs[:, 4:5]
            it16 = c16[:, 0 : CBmax * W]
            dstrel_s = c16[:, CBmax * W : CBmax * W + NCH]
            oall = cpool.tile([P, NPC], f16, tag="oall")

            # group sizes: GPW windows each, tapering at the tail to
            # shorten the pipeline drain (last windows are also the
            # smallest thanks to the descending-count permutation)
            gsizes = _group_sizes()
            gstart = [0]
            for s in gsizes:
                gstart.append(gstart[-1] + s)

            for j in range(WINDOWS):
                cb = CB[j]
                off = int(offs[j])
                g = next(i for i in range(len(gsizes)) if gstart[i + 1] > j)
                sw = j - gstart[g]
                gsz = gsizes[g]

                if sw == 0:
                    # one combined edge-feature + node-feature DMA per group
                    goff = off
                    gend = int(offs[gstart[g + 1]])
                    gw = (gend - goff) * D + gsz * W
                    gsrc = goff * D + gstart[g] * W
                    at = apool.tile([P, GPW * (CBmax * D + W)], f16, tag="attr")
                    nc.sync.dma_start(
                        out=at[:, :gw], in_=attrT_d[:, gsrc : gsrc + gw]
                    )
                    if j == 0:
                        nc.sync.dma_start(out=c16[:], in_=c16_d[:, :])
                        nc.sync.dma_start(out=cs[:], in_=consts_d[:, :])
                        nc.sync.dma_start(out=ws[:], in_=wts_d[:, :])
                    # flush the previous group's finished output slice
                    if g > 0 and gsizes[g - 1] == GPW:
                        f0, f1 = gstart[g - 1] * W, gstart[g] * W
                        nc.sync.dma_start(
                            out=outT_d[:, f0:f1], in_=oall[:, f0:f1]
                        )
                woff = off - goff  # window's chunk offset within group tile

                oh = ohpool.tile([P, CBmax * W], f16, tag="oh")
                if "oh" not in ablate:
                    nc.vector.tensor_tensor(
                        out=oh[:, : cb * W].rearrange("p (c m) -> p c m", m=W),
                        in0=dstrel_s[:, off : off + cb].to_broadcast([P, cb, W]),
                        in1=it16[:, : cb * W].rearrange("p (c m) -> p c m", m=W),
                        op=mybir.AluOpType.is_equal,
                    )

                pm = pbin.tile([P, W], f32, tag="mean")
                for ch in range(cb if "bin" not in ablate else 0):
                    nc.tensor.matmul(
                        out=pm[:],
                        lhsT=at[:, (woff + ch) * D : (woff + ch + 1) * D],
                        rhs=oh[:, ch * W : (ch + 1) * W],
                        start=(ch == 0),
                        stop=(ch == cb - 1),
                    )

                if sw == 0:
                    mean_g = actpool.tile([P, GROUP_N], f16, tag="mean_g")
                if "bin" not in ablate:
                    nc.scalar.copy(out=mean_g[:, sw * W : (sw + 1) * W], in_=pm[:])

                if ("mlp" not in ablate) and (sw == gsz - 1):
                    # --- MLP over this group of nodes (feature-major) ---
                    NW = gsz * W
                    n0 = gstart[g] * W

                    ph1a = pmlp.tile([P, GROUP_N], f32, tag="h1a")
                    ph1b = pmlp.tile([P, GROUP_N], f32, tag="h1b")
                    nc.tensor.matmul(out=ph1a[:, :NW], lhsT=w1s_0[:, 0:P],
                                     rhs=at[:, (gend - goff) * D : (gend - goff) * D + NW], start=True, stop=False)
                    nc.tensor.matmul(out=ph1a[:, :NW], lhsT=w1s_1[:, 0:P],
                                     rhs=mean_g[:, :NW], start=False, stop=True)
                    nc.tensor.matmul(out=ph1b[:, :NW], lhsT=w1s_0[:, P:HIDDEN],
                                     rhs=at[:, (gend - goff) * D : (gend - goff) * D + NW], start=True, stop=False)
                    nc.tensor.matmul(out=ph1b[:, :NW], lhsT=w1s_1[:, P:HIDDEN],
                                     rhs=mean_g[:, :NW], start=False, stop=True)
                    h1a = actpool.tile([P, GROUP_N], f16, tag="h1a_s")
                    h1b = actpool.tile([P, GROUP_N], f16, tag="h1b_s")
                    nc.scalar.activation(out=h1a[:, :NW], in_=ph1a[:, :NW],
                                         func=Relu, bias=b1s_0[:, 0:1])
                    nc.scalar.activation(out=h1b[:, :NW], in_=ph1b[:, :NW],
                                         func=Relu, bias=b1s_1[:, 0:1])

                    ph2a = pmlp.tile([P, GROUP_N], f32, tag="h2a")
                    ph2b = pmlp.tile([P, GROUP_N], f32, tag="h2b")
                    nc.tensor.matmul(out=ph2a[:, :NW], lhsT=w2s_0[:, 0:P],
                                     rhs=h1a[:, :NW], start=True, stop=False)
                    nc.tensor.matmul(out=ph2a[:, :NW], lhsT=w2s_1[:, 0:P],
                                     rhs=h1b[:, :NW], start=False, stop=True)
                    nc.tensor.matmul(out=ph2b[:, :NW], lhsT=w2s_0[:, P:HIDDEN],
                                     rhs=h1a[:, :NW], start=True, stop=False)
                    nc.tensor.matmul(out=ph2b[:, :NW], lhsT=w2s_1[:, P:HIDDEN],
                                     rhs=h1b[:, :NW], start=False, stop=True)
                    h2a = actpool.tile([P, GROUP_N], f16, tag="h2a_s")
                    h2b = actpool.tile([P, GROUP_N], f16, tag="h2b_s")
                    nc.scalar.activation(out=h2a[:, :NW], in_=ph2a[:, :NW],
                                         func=Relu, bias=b2s_0[:, 0:1])
                    nc.scalar.activation(out=h2b[:, :NW], in_=ph2b[:, :NW],
                                         func=Relu, bias=b2s_1[:, 0:1])

                    po = pmlp.tile([P, GROUP_N], f32, tag="o")
                    nc.tensor.matmul(out=po[:, :NW], lhsT=w3s_0[:],
                                     rhs=h2a[:, :NW], start=True, stop=False)
                    nc.tensor.matmul(out=po[:, :NW], lhsT=w3s_1[:],
                                     rhs=h2b[:, :NW], start=False, stop=True)
                    nc.scalar.activation(out=oall[:, n0 : n0 + NW],
                                         in_=po[:, :NW],
                                         func=Ident, bias=b3s[:, 0:1])
                    if gsz < GPW:
                        # tail taper groups: no more prefetches to protect,
                        # store immediately to shorten the drain
                        nc.sync.dma_start(
                            out=outT_d[:, n0 : n0 + NW], in_=oall[:, n0 : n0 + NW]
                        )

            if gsizes[-1] == GPW:
                f0 = gstart[len(gsizes) - 1] * W
                nc.sync.dma_start(out=outT_d[:, f0:], in_=oall[:, f0:])

    # run_bass_via_pjrt (axon path) does not finalize; Bacc needs
    # finalize() to run its compile passes (reg alloc, wait legalization).
    nc.finalize()
    return nc


def _host_prep(x, edge_index, edge_attr):
    """Sort/scale/pad edges; returns (CB, per-core input arrays)."""
    col = np.asarray(edge_index)[1].astype(np.int64)
    x = np.asarray(x, dtype=np.float32)
    counts = np.bincount(col, minlength=N_NODES)
    scale = (1.0 / np.maximum(counts, 1)).astype(np.float32)

    order = np.argsort(col, kind="stable")
    col_s = col[order]
    attr_s = np.asarray(edge_attr, dtype=np.float32)[order]
    attr_s = attr_s * scale[col_s][:, None]

    # per-core, per-window edge counts
    starts = np.empty((N_CORES, WINDOWS + 1), dtype=np.int64)
    for c in range(N_CORES):
        bounds = np.minimum(
            c * NPC_REAL + np.arange(WINDOWS + 1) * W, (c + 1) * NPC_REAL
        )
        starts[c] = np.searchsorted(col_s, bounds)
    cnt = np.diff(starts, axis=1)  # [N_CORES, WINDOWS]

    # Each core processes its windows sorted by descending edge count.
    # Window slot j then holds every core's j-th order statistic, so the
    # cross-core max (CB must be shared, the program is SPMD) wastes far
    # less padding than positional assignment.  Small windows land last,
    # which also shortens the pipeline drain.  Host un-permutes outputs.
    order = np.argsort(-cnt, axis=1, kind="stable")  # [N_CORES, WINDOWS]
    cnt_s = np.take_along_axis(cnt, order, axis=1)

    CB = np.maximum(1, (-(-cnt_s // P)).max(axis=0)).astype(int)  # ceil, >=1
    offs = np.concatenate([[0], np.cumsum(CB)]).astype(np.int64)
    NCH = int(offs[-1])
    E_pad = NCH * P

    per_core = []
    for c in range(N_CORES):
        ordc = order[c]
        cnts = cnt_s[c]                      # counts in processing order
        total = int(cnts.sum())
        # edge source rows (into col_s/attr_s), in processing order
        src_idx = np.concatenate(
            [np.arange(starts[c, w], starts[c, w + 1]) for w in ordc]
        )
        base = np.repeat(offs[:-1] * P, cnts)
        within = np.arange(total) - np.repeat(np.cumsum(cnts) - cnts, cnts)
        edest = base + within

        attr_pad = np.zeros((E_pad, D), np.float32)
        attr_pad[edest] = attr_s[src_idx]
        attrT_edges = (
            attr_pad.reshape(NCH, P, D)
            .transpose(1, 0, 2)
            .reshape(P, NCH * D)
            .astype(np.float16)
        )

        # dst relative to the processed window's node base
        win_base_proc = c * NPC_REAL + ordc * W  # global node base per slot
        dstrel = np.full((E_pad,), 200.0, np.float16)
        dstrel[edest] = (
            col_s[src_idx] - np.repeat(win_base_proc, cnts)
        ).astype(np.float16)
        dstrelT = np.ascontiguousarray(dstrel.reshape(NCH, P).T)

        # node features per 64-node window slot, zero-padded per slot
        xc = np.zeros((WINDOWS, W, D), np.float16)
        for j, w in enumerate(ordc):
            n0 = c * NPC_REAL + w * W
            n1 = min(n0 + W, (c + 1) * NPC_REAL)
            xc[j, : n1 - n0] = x[n0:n1].astype(np.float16)
        xT = xc.reshape(NPC, D).T  # [D, NPC]

        # interleave per group: [edge chunks | node features]
        gsizes = _group_sizes()
        attrT = np.empty((P, NCH * D + WINDOWS * W), np.float16)
        pos = 0
        j0 = 0
        for gsz in gsizes:
            c0, c1 = int(offs[j0]), int(offs[j0 + gsz])
            wgt = (c1 - c0) * D
            attrT[:, pos : pos + wgt] = attrT_edges[:, c0 * D : c1 * D]
            pos += wgt
            attrT[:, pos : pos + gsz * W] = xT[:, j0 * W : (j0 + gsz) * W]
            pos += gsz * W
            j0 += gsz
        assert pos == attrT.shape[1] and j0 == WINDOWS

        per_core.append(
            {"attrT": np.ascontiguousarray(attrT), "dstrelT": dstrelT,
             "order": ordc}
        )
    return tuple(CB.tolist()), per_core


def _build_consts(b1, b2, b3):
    consts = np.zeros((P, 5), np.float32)
    consts[:, 0] = b1[:P]
    consts[:, 1] = b1[P:]
    consts[:, 2] = b2[:P]
    consts[:, 3] = b2[P:]
    consts[:, 4] = b3
    return consts


def _build_wts(W1, W2, W3):
    wts = np.empty((P, 4 * HIDDEN + 2 * DOUT), np.float16)
    wts[:, 0:HIDDEN] = W1[:P]
    wts[:, HIDDEN : 2 * HIDDEN] = W1[P:]
    wts[:, 2 * HIDDEN : 3 * HIDDEN] = W2[:P]
    wts[:, 3 * HIDDEN : 4 * HIDDEN] = W2[P:]
    wts[:, 4 * HIDDEN : 4 * HIDDEN + DOUT] = W3[:P]
    wts[:, 4 * HIDDEN + DOUT : 4 * HIDDEN + 2 * DOUT] = W3[P:]
    return wts


def _build_c16(CB, dstrelT):
    """fp16 consts row-block: iota ramp | dstrel."""
    CBmax = max(CB)
    NCH = int(sum(CB))
    c16 = np.empty((P, CBmax * W + NCH), np.float16)
    c16[:, 0 : CBmax * W] = np.tile(np.arange(W, dtype=np.float16), CBmax)[None, :]
    c16[:, CBmax * W :] = dstrelT
    return c16


def kernel(x, edge_index, edge_attr, W1, b1, W2, b2, W3, b3):
    CB, per_core = _host_prep(x, edge_index, edge_attr)

    key = CB
    if key not in _prog_cache:
        _prog_cache[key] = _build_program(CB)
    nc = _prog_cache[key]

    W1 = np.asarray(W1, np.float32)
    W2 = np.asarray(W2, np.float32)
    W3 = np.asarray(W3, np.float32)
    b1 = np.asarray(b1, np.float32)
    b2 = np.asarray(b2, np.float32)
    b3 = np.asarray(b3, np.float32)
    consts = _build_consts(b1, b2, b3)
    wts = _build_wts(W1, W2, W3)
    in_maps = [
        {
            "attrT": pc["attrT"],
            "c16": _build_c16(CB, pc["dstrelT"]),
            "consts": consts,
            "wts": wts,
        }
        for pc in per_core
    ]

    res = run_bass_kernel_spmd(nc, in_maps, core_ids=list(range(N_CORES)))

    out = np.empty((N_NODES, DOUT), np.float32)
    for c in range(N_CORES):
        o = res.results[c]["outT"].T.astype(np.float32).reshape(WINDOWS, W, DOUT)
        for j, w in enumerate(per_core[c]["order"]):
            n0 = c * NPC_REAL + int(w) * W
            n1 = min(n0 + W, (c + 1) * NPC_REAL)
            out[n0:n1] = o[j, : n1 - n0]
    return out


# revision 43
# speedup vs baseline: 1.0067x; 1.0067x over previous
"""NodeNet GNN message-passing kernel for 8 Trainium2 NeuronCores.

Strategy (per sharding hint): shard nodes across the 8 cores; partition
edges by destination node on the host so the scatter-mean is device-local.

Per core (12,500 real nodes, padded to 12,544 = 196 windows of 64 nodes):
  - Host sorts edges by destination and pre-scales each edge row by
    1/count(dst), so the device segment-sum directly yields the mean.
    Each 64-node window's edge list is padded to a multiple of 128; each
    core processes its windows in descending-edge-count order so the
    shared (SPMD) per-window chunk counts CB[j] = max-over-cores of the
    j-th order statistic waste minimal padding, and the smallest windows
    land at the end, shortening the pipeline drain.  Edge features are
    laid out chunk-transposed fp16 ([128, nch*128]), with each MLP
    group's node features interleaved into the same stream, so the whole
    input arrives as one wide contiguous DMA per group.
  - Device builds, per 128-edge chunk, a [128 edge, 64 node] fp16
    one-hot (is_equal of dst-rel against an iota ramp, VectorE) and
    contracts it on the TensorEngine:
    meanT[d, n] += matmul(lhsT=attr[e, d], rhs=onehot[e, n]) accumulated
    in PSUM (fp32).  Everything stays feature-major so the 3-layer MLP
    (fp16 matmuls, fp32 PSUM accumulate, ScalarE relu+bias evacuations)
    chains with no transposes: h1T = relu(W1.T @ [xT; meanT] + b1), ...
  - Output is accumulated feature-major fp16 in SBUF and stored with one
    deferred DMA per group; the host transposes, upcasts, and un-permutes.

Cost-model timeline (per core): ~191 us, ~90% of the 173 us DMA-byte
floor (~62 MB/core at ~360 GB/s); VectorE/ScalarE/TensorE all at or
below ~65% occupancy, fully hidden behind the edge-feature stream.
"""

import numpy as np

import concourse.bacc as bacc
import concourse.mybir as mybir
import concourse.tile as tile
from concourse.bass_utils import run_bass_kernel_spmd

P = 128                    # partitions / matmul contraction tile
D = 128                    # node & edge feature dim
HIDDEN = 256
DOUT = 128
N_NODES = 100000
N_CORES = 8
NPC_REAL = 12500           # real nodes per core
W = 64                     # nodes per binning window
WINDOWS = 196              # windows per core (196*64 = 12544)
NPC = WINDOWS * W          # padded nodes per core
GPW = 8                    # windows per MLP group (512 nodes)
GROUP_N = GPW * W
ATTR_BUFS = 3
OH_BUFS = 6
ACT_BUFS = 4
PBIN_BUFS = 3

_prog_cache: dict = {}

f32 = mybir.dt.float32
f16 = mybir.dt.float16


def _group_sizes():
    gsizes = []
    rem = WINDOWS
    while rem > GPW:
        gsizes.append(GPW)
        rem -= GPW
    while rem > 0:
        t = min(GPW // 2, rem)
        gsizes.append(t)
        rem -= t
    return gsizes


def _build_program(CB, ablate=()):
    """Build the Bass/Tile program. CB[j] = number of 128-edge chunks for
    window j (identical across cores; per-core data is padded to match).
    ablate: subset of {"mlp", "bin", "oh"} to skip stages (sim studies)."""
    CB = list(CB)
    CBmax = max(CB)
    offs = np.concatenate([[0], np.cumsum(CB)]).astype(int)
    NCH = int(offs[-1])

    nc = bacc.Bacc(None)
    # attrT carries, per group: the edge-feature chunks, then the group's
    # node features (gsz*W fp16 columns) — one combined DMA per group.
    attrT_d = nc.dram_tensor(
        "attrT", [P, NCH * D + WINDOWS * W], f16, kind="ExternalInput"
    )
    # fp16 consts: iota ramp (CBmax*W) | dstrel (NCH)
    c16_d = nc.dram_tensor("c16", [P, CBmax * W + NCH], f16, kind="ExternalInput")
    # fp32 consts: 5 bias columns
    consts_d = nc.dram_tensor("consts", [P, 5], f32, kind="ExternalInput")
    wts_d = nc.dram_tensor("wts", [P, 4 * HIDDEN + 2 * DOUT], f16,
                           kind="ExternalInput")
    outT_d = nc.dram_tensor("outT", [P, NPC], f16, kind="ExternalOutput")

    Relu = mybir.ActivationFunctionType.Relu
    Ident = mybir.ActivationFunctionType.Identity

    with tile.TileContext(nc) as tc:
        with (
            tc.tile_pool(name="const", bufs=1) as cpool,
            tc.tile_pool(name="attr", bufs=ATTR_BUFS) as apool,
            tc.tile_pool(name="oh", bufs=OH_BUFS) as ohpool,
            tc.tile_pool(name="acts", bufs=ACT_BUFS) as actpool,
            tc.tile_pool(name="pbin", bufs=PBIN_BUFS, space="PSUM") as pbin,
            tc.tile_pool(name="pmlp", bufs=1, space="PSUM") as pmlp,
        ):
            # --- constants (tiles now; DMAs after the first attr DMA so
            # the edge stream starts immediately) ---
            cs = cpool.tile([P, 5], f32, tag="consts")
            ws = cpool.tile([P, 4 * HIDDEN + 2 * DOUT], f16, tag="wts")
            c16 = cpool.tile([P, CBmax * W + NCH], f16, tag="c16")
            w1s_0 = ws[:, 0:HIDDEN]
            w1s_1 = ws[:, HIDDEN : 2 * HIDDEN]
            w2s_0 = ws[:, 2 * HIDDEN : 3 * HIDDEN]
            w2s_1 = ws[:, 3 * HIDDEN : 4 * HIDDEN]
            w3s_0 = ws[:, 4 * HIDDEN : 4 * HIDDEN + DOUT]
            w3s_1 = ws[:, 4 * HIDDEN + DOUT : 4 * HIDDEN + 2 * DOUT]
            b1s_0 = cs[:, 0:1]
            b1s_1 = cs[:, 1:2]
            b2s_0 = cs[:, 2:3]
            b2s_1 = cs[:, 3:4]
            b3s = cs[:, 4:5]
            it16 = c16[:, 0 : CBmax * W]
            dstrel_s = c16[:, CBmax * W : CBmax * W + NCH]
            oall = cpool.tile([P, NPC], f16, tag="oall")

            # group sizes: GPW windows each, tapering at the tail to
            # shorten the pipeline drain (last windows are also the
            # smallest thanks to the descending-count permutation)
            gsizes = _group_sizes()
            gstart = [0]
            for s in gsizes:
                gstart.append(gstart[-1] + s)

            for j in range(WINDOWS):
                cb = CB[j]
                off = int(offs[j])
                g = next(i for i in range(len(gsizes)) if gstart[i + 1] > j)
                sw = j - gstart[g]
                gsz = gsizes[g]

                if sw == 0:
                    # one combined edge-feature + node-feature DMA per group
                    goff = off
                    gend = int(offs[gstart[g + 1]])
                    gw = (gend - goff) * D + gsz * W
                    gsrc = goff * D + gstart[g] * W
                    at = apool.tile([P, GPW * (CBmax * D + W)], f16, tag="attr")
                    nc.sync.dma_start(
                        out=at[:, :gw], in_=attrT_d[:, gsrc : gsrc + gw]
                    )
                    if j == 0:
                        nc.sync.dma_start(out=c16[:], in_=c16_d[:, :])
                        nc.sync.dma_start(out=cs[:], in_=consts_d[:, :])
                        nc.sync.dma_start(out=ws[:], in_=wts_d[:, :])
                    # flush the previous group's finished output slice
                    if g > 0 and gsizes[g - 1] == GPW:
                        f0, f1 = gstart[g - 1] * W, gstart[g] * W
                        nc.sync.dma_start(
                            out=outT_d[:, f0:f1], in_=oall[:, f0:f1]
                        )
                woff = off - goff  # window's chunk offset within group tile

                oh = ohpool.tile([P, CBmax * W], f16, tag="oh")
                if "oh" not in ablate:
                    nc.vector.tensor_tensor(
                        out=oh[:, : cb * W].rearrange("p (c m) -> p c m", m=W),
                        in0=dstrel_s[:, off : off + cb].to_broadcast([P, cb, W]),
                        in1=it16[:, : cb * W].rearrange("p (c m) -> p c m", m=W),
                        op=mybir.AluOpType.is_equal,
                    )

                pm = pbin.tile([P, W], f32, tag="mean")
                for ch in range(cb if "bin" not in ablate else 0):
                    nc.tensor.matmul(
                        out=pm[:],
                        lhsT=at[:, (woff + ch) * D : (woff + ch + 1) * D],
                        rhs=oh[:, ch * W : (ch + 1) * W],
                        start=(ch == 0),
                        stop=(ch == cb - 1),
                    )

                if sw == 0:
                    mean_g = actpool.tile([P, GROUP_N], f16, tag="mean_g")
                if "bin" not in ablate:
                    nc.scalar.copy(out=mean_g[:, sw * W : (sw + 1) * W], in_=pm[:])

                if ("mlp" not in ablate) and (sw == gsz - 1):
                    # --- MLP over this group of nodes (feature-major) ---
                    NW = gsz * W
                    n0 = gstart[g] * W

                    ph1a = pmlp.tile([P, GROUP_N], f32, tag="h1a")
                    ph1b = pmlp.tile([P, GROUP_N], f32, tag="h1b")
                    nc.tensor.matmul(out=ph1a[:, :NW], lhsT=w1s_0[:, 0:P],
                                     rhs=at[:, (gend - goff) * D : (gend - goff) * D + NW], start=True, stop=False)
                    nc.tensor.matmul(out=ph1a[:, :NW], lhsT=w1s_1[:, 0:P],
                                     rhs=mean_g[:, :NW], start=False, stop=True)
                    nc.tensor.matmul(out=ph1b[:, :NW], lhsT=w1s_0[:, P:HIDDEN],
                                     rhs=at[:, (gend - goff) * D : (gend - goff) * D + NW], start=True, stop=False)
                    nc.tensor.matmul(out=ph1b[:, :NW], lhsT=w1s_1[:, P:HIDDEN],
                                     rhs=mean_g[:, :NW], start=False, stop=True)
                    h1a = actpool.tile([P, GROUP_N], f16, tag="h1a_s")
                    h1b = actpool.tile([P, GROUP_N], f16, tag="h1b_s")
                    nc.scalar.activation(out=h1a[:, :NW], in_=ph1a[:, :NW],
                                         func=Relu, bias=b1s_0[:, 0:1])
                    nc.scalar.activation(out=h1b[:, :NW], in_=ph1b[:, :NW],
                                         func=Relu, bias=b1s_1[:, 0:1])

                    ph2a = pmlp.tile([P, GROUP_N], f32, tag="h2a")
                    ph2b = pmlp.tile([P, GROUP_N], f32, tag="h2b")
                    nc.tensor.matmul(out=ph2a[:, :NW], lhsT=w2s_0[:, 0:P],
                                     rhs=h1a[:, :NW], start=True, stop=False)
                    nc.tensor.matmul(out=ph2a[:, :NW], lhsT=w2s_1[:, 0:P],
                                     rhs=h1b[:, :NW], start=False, stop=True)
                    nc.tensor.matmul(out=ph2b[:, :NW], lhsT=w2s_0[:, P:HIDDEN],
                                     rhs=h1a[:, :NW], start=True, stop=False)
                    nc.tensor.matmul(out=ph2b[:, :NW], lhsT=w2s_1[:, P:HIDDEN],
                                     rhs=h1b[:, :NW], start=False, stop=True)
                    h2a = actpool.tile([P, GROUP_N], f16, tag="h2a_s")
                    h2b = actpool.tile([P, GROUP_N], f16, tag="h2b_s")
                    nc.scalar.activation(out=h2a[:, :NW], in_=ph2a[:, :NW],
                                         func=Relu, bias=b2s_0[:, 0:1])
                    nc.scalar.activation(out=h2b[:, :NW], in_=ph2b[:, :NW],
                                         func=Relu, bias=b2s_1[:, 0:1])

                    po = pmlp.tile([P, GROUP_N], f32, tag="o")
                    nc.tensor.matmul(out=po[:, :NW], lhsT=w3s_0[:],
                                     rhs=h2a[:, :NW], start=True, stop=False)
                    nc.tensor.matmul(out=po[:, :NW], lhsT=w3s_1[:],
                                     rhs=h2b[:, :NW], start=False, stop=True)
                    nc.scalar.activation(out=oall[:, n0 : n0 + NW],
                                         in_=po[:, :NW],
                                         func=Ident, bias=b3s[:, 0:1])
                    if gsz < GPW:
                        # tail taper groups: no more prefetches to protect,
                        # store immediately to shorten the drain
                        nc.sync.dma_start(
                            out=outT_d[:, n0 : n0 + NW], in_=oall[:, n0 : n0 + NW]
                        )

            if gsizes[-1] == GPW:
                f0 = gstart[len(gsizes) - 1] * W
                nc.sync.dma_start(out=outT_d[:, f0:], in_=oall[:, f0:])

    # run_bass_via_pjrt (axon path) does not finalize; Bacc needs
    # finalize() to run its compile passes (reg alloc, wait legalization).
    nc.finalize()
    return nc


def _host_prep(x, edge_index, edge_attr):
    """Sort/scale/pad edges; returns (CB, per-core input arrays)."""
    col = np.asarray(edge_index)[1].astype(np.int64)
    x = np.asarray(x, dtype=np.float32)
    counts = np.bincount(col, minlength=N_NODES)
    scale = (1.0 / np.maximum(counts, 1)).astype(np.float32)

    order = np.argsort(col, kind="stable")
    col_s = col[order]
    attr_s = np.asarray(edge_attr, dtype=np.float32)[order]
    attr_s = attr_s * scale[col_s][:, None]

    # per-core, per-window edge counts
    starts = np.empty((N_CORES, WINDOWS + 1), dtype=np.int64)
    for c in range(N_CORES):
        bounds = np.minimum(
            c * NPC_REAL + np.arange(WINDOWS + 1) * W, (c + 1) * NPC_REAL
        )
        starts[c] = np.searchsorted(col_s, bounds)
    cnt = np.diff(starts, axis=1)  # [N_CORES, WINDOWS]

    # Each core processes its windows sorted by descending edge count.
    # Window slot j then holds every core's j-th order statistic, so the
    # cross-core max (CB must be shared, the program is SPMD) wastes far
    # less padding than positional assignment.  Small windows land last,
    # which also shortens the pipeline drain.  Host un-permutes outputs.
    order = np.argsort(-cnt, axis=1, kind="stable")  # [N_CORES, WINDOWS]
    cnt_s = np.take_along_axis(cnt, order, axis=1)

    CB = np.maximum(1, (-(-cnt_s // P)).max(axis=0)).astype(int)  # ceil, >=1
    offs = np.concatenate([[0], np.cumsum(CB)]).astype(np.int64)
    NCH = int(offs[-1])
    E_pad = NCH * P

    per_core = []
    for c in range(N_CORES):
        ordc = order[c]
        cnts = cnt_s[c]                      # counts in processing order
        total = int(cnts.sum())
        # edge source rows (into col_s/attr_s), in processing order
        src_idx = np.concatenate(
            [np.arange(starts[c, w], starts[c, w + 1]) for w in ordc]
        )
        base = np.repeat(offs[:-1] * P, cnts)
        within = np.arange(total) - np.repeat(np.cumsum(cnts) - cnts, cnts)
        edest = base + within

        attr_pad = np.zeros((E_pad, D), np.float32)
        attr_pad[edest] = attr_s[src_idx]
        attrT_edges = (
            attr_pad.reshape(NCH, P, D)
            .transpose(1, 0, 2)
            .reshape(P, NCH * D)
            .astype(np.float16)
        )

        # dst relative to the processed window's node base
        win_base_proc = c * NPC_REAL + ordc * W  # global node base per slot
        dstrel = np.full((E_pad,), 200.0, np.float16)
        dstrel[edest] = (
            col_s[src_idx] - np.repeat(win_base_proc, cnts)
        ).astype(np.float16)
        dstrelT = np.ascontiguousarray(dstrel.reshape(NCH, P).T)

        # node features per 64-node window slot, zero-padded per slot
        xc = np.zeros((WINDOWS, W, D), np.float16)
        for j, w in enumerate(ordc):
            n0 = c * NPC_REAL + w * W
            n1 = min(n0 + W, (c + 1) * NPC_REAL)
            xc[j, : n1 - n0] = x[n0:n1].astype(np.float16)
        xT = xc.reshape(NPC, D).T  # [D, NPC]

        # interleave per group: [edge chunks | node features]
        gsizes = _group_sizes()
        attrT = np.empty((P, NCH * D + WINDOWS * W), np.float16)
        pos = 0
        j0 = 0
        for gsz in gsizes:
            c0, c1 = int(offs[j0]), int(offs[j0 + gsz])
            wgt = (c1 - c0) * D
            attrT[:, pos : pos + wgt] = attrT_edges[:, c0 * D : c1 * D]
            pos += wgt
            attrT[:, pos : pos + gsz * W] = xT[:, j0 * W : (j0 + gsz) * W]
            pos += gsz * W
            j0 += gsz
        assert pos == attrT.shape[1] and j0 == WINDOWS

        per_core.append(
            {"attrT": np.ascontiguousarray(attrT), "dstrelT": dstrelT,
             "order": ordc}
        )
    return tuple(CB.tolist()), per_core


def _build_consts(b1, b2, b3):
    consts = np.zeros((P, 5), np.float32)
    consts[:, 0] = b1[:P]
    consts[:, 1] = b1[P:]
    consts[:, 2] = b2[:P]
    consts[:, 3] = b2[P:]
    consts[:, 4] = b3
    return consts


def _build_wts(W1, W2, W3):
    wts = np.empty((P, 4 * HIDDEN + 2 * DOUT), np.float16)
    wts[:, 0:HIDDEN] = W1[:P]
    wts[:, HIDDEN : 2 * HIDDEN] = W1[P:]
    wts[:, 2 * HIDDEN : 3 * HIDDEN] = W2[:P]
    wts[:, 3 * HIDDEN : 4 * HIDDEN] = W2[P:]
    wts[:, 4 * HIDDEN : 4 * HIDDEN + DOUT] = W3[:P]
    wts[:, 4 * HIDDEN + DOUT : 4 * HIDDEN + 2 * DOUT] = W3[P:]
    return wts


def _build_c16(CB, dstrelT):
    """fp16 consts row-block: iota ramp | dstrel."""
    CBmax = max(CB)
    NCH = int(sum(CB))
    c16 = np.empty((P, CBmax * W + NCH), np.float16)
    c16[:, 0 : CBmax * W] = np.tile(np.arange(W, dtype=np.float16), CBmax)[None, :]
    c16[:, CBmax * W :] = dstrelT
    return c16


def kernel(x, edge_index, edge_attr, W1, b1, W2, b2, W3, b3):
    CB, per_core = _host_prep(x, edge_index, edge_attr)

    key = CB
    if key not in _prog_cache:
        _prog_cache[key] = _build_program(CB)
    nc = _prog_cache[key]

    W1 = np.asarray(W1, np.float32)
    W2 = np.asarray(W2, np.float32)
    W3 = np.asarray(W3, np.float32)
    b1 = np.asarray(b1, np.float32)
    b2 = np.asarray(b2, np.float32)
    b3 = np.asarray(b3, np.float32)
    consts = _build_consts(b1, b2, b3)
    wts = _build_wts(W1, W2, W3)
    in_maps = [
        {
            "attrT": pc["attrT"],
            "c16": _build_c16(CB, pc["dstrelT"]),
            "consts": consts,
            "wts": wts,
        }
        for pc in per_core
    ]

    res = run_bass_kernel_spmd(nc, in_maps, core_ids=list(range(N_CORES)))

    out = np.empty((N_NODES, DOUT), np.float32)
    for c in range(N_CORES):
        o = res.results[c]["outT"].T.astype(np.float32).reshape(WINDOWS, W, DOUT)
        for j, w in enumerate(per_core[c]["order"]):
            n0 = c * NPC_REAL + int(w) * W
            n1 = min(n0 + W, (c + 1) * NPC_REAL)
            out[n0:n1] = o[j, : n1 - n0]
    return out


# revision 44
# speedup vs baseline: 1.0073x; 1.0006x over previous
"""NodeNet GNN message-passing kernel for 8 Trainium2 NeuronCores.

Strategy (per sharding hint): shard nodes across the 8 cores; partition
edges by destination node on the host so the scatter-mean is device-local.

Per core (12,500 real nodes, padded to 12,544 = 196 windows of 64 nodes):
  - Host sorts edges by destination and pre-scales each edge row by
    1/count(dst), so the device segment-sum directly yields the mean.
    Each 64-node window's edge list is padded to a multiple of 128; each
    core processes its windows in descending-edge-count order so the
    shared (SPMD) per-window chunk counts CB[j] = max-over-cores of the
    j-th order statistic waste minimal padding, and the smallest windows
    land at the end, shortening the pipeline drain.  Edge features are
    laid out chunk-transposed fp16 ([128, nch*128]), with each MLP
    group's node features interleaved into the same stream, so the whole
    input arrives as one wide contiguous DMA per group.
  - Device builds, per 128-edge chunk, a [128 edge, 64 node] fp16
    one-hot (is_equal of dst-rel against an iota ramp, VectorE) and
    contracts it on the TensorEngine:
    meanT[d, n] += matmul(lhsT=attr[e, d], rhs=onehot[e, n]) accumulated
    in PSUM (fp32).  Everything stays feature-major so the 3-layer MLP
    (fp16 matmuls, fp32 PSUM accumulate, ScalarE relu+bias evacuations)
    chains with no transposes: h1T = relu(W1.T @ [xT; meanT] + b1), ...
  - Windows whose (cross-core max) edge-count remainder fits in 64 edges
    pair up inside each group: two tails share one 128-row chunk (one in
    partitions 0:64, the other in 64:128, contracted by two K=64
    matmuls), trimming most of the chunk-quantization padding.
  - Output is accumulated feature-major fp16 in SBUF and stored with one
    deferred DMA per group; the host transposes, upcasts, and un-permutes.

Cost-model timeline (per core): ~190 us against a ~169 us DMA-byte
floor (~61 MB/core at ~360 GB/s); VectorE/ScalarE/TensorE all at or
below ~65% occupancy, fully hidden behind the edge-feature stream.
"""

import numpy as np

import concourse.bacc as bacc
import concourse.mybir as mybir
import concourse.tile as tile
from concourse.bass_utils import run_bass_kernel_spmd

P = 128                    # partitions / matmul contraction tile
D = 128                    # node & edge feature dim
HIDDEN = 256
DOUT = 128
N_NODES = 100000
N_CORES = 8
NPC_REAL = 12500           # real nodes per core
W = 64                     # nodes per binning window
WINDOWS = 196              # windows per core (196*64 = 12544)
NPC = WINDOWS * W          # padded nodes per core
GPW = 8                    # windows per MLP group (512 nodes)
GROUP_N = GPW * W
ATTR_BUFS = 3
OH_BUFS = 6
ACT_BUFS = 4
PBIN_BUFS = 3

_prog_cache: dict = {}

f32 = mybir.dt.float32
f16 = mybir.dt.float16


def _group_sizes():
    gsizes = []
    rem = WINDOWS
    while rem > GPW:
        gsizes.append(GPW)
        rem -= GPW
    while rem > 0:
        t = min(GPW // 2, rem)
        gsizes.append(t)
        rem -= t
    return gsizes


def _build_program(META, ablate=()):
    """Build the Bass/Tile program. META = (NCH, per-window tuples of
    (col_off, ncols, fullc, tailmode)) — identical across cores.
    tailmode: 0 = all-full chunks; 1 = last chunk is a shared half
    (partitions 0:64); 2 = first chunk is a shared half (partitions
    64:128).  ablate: subset of {"mlp", "bin", "oh"} (sim studies)."""
    NCH, wmeta = META
    col_off = [m[0] for m in wmeta]
    ncols = [m[1] for m in wmeta]
    CBmax = max(ncols)

    nc = bacc.Bacc(None)
    # attrT carries, per group: the edge-feature chunks, then the group's
    # node features (gsz*W fp16 columns) — one combined DMA per group.
    attrT_d = nc.dram_tensor(
        "attrT", [P, NCH * D + WINDOWS * W], f16, kind="ExternalInput"
    )
    # fp16 consts: iota ramp (CBmax*W) | dstrel (NCH)
    c16_d = nc.dram_tensor("c16", [P, CBmax * W + NCH], f16, kind="ExternalInput")
    # fp32 consts: 5 bias columns
    consts_d = nc.dram_tensor("consts", [P, 5], f32, kind="ExternalInput")
    wts_d = nc.dram_tensor("wts", [P, 4 * HIDDEN + 2 * DOUT], f16,
                           kind="ExternalInput")
    outT_d = nc.dram_tensor("outT", [P, NPC], f16, kind="ExternalOutput")

    Relu = mybir.ActivationFunctionType.Relu
    Ident = mybir.ActivationFunctionType.Identity

    with tile.TileContext(nc) as tc:
        with (
            tc.tile_pool(name="const", bufs=1) as cpool,
            tc.tile_pool(name="attr", bufs=ATTR_BUFS) as apool,
            tc.tile_pool(name="oh", bufs=OH_BUFS) as ohpool,
            tc.tile_pool(name="acts", bufs=ACT_BUFS) as actpool,
            tc.tile_pool(name="pbin", bufs=PBIN_BUFS, space="PSUM") as pbin,
            tc.tile_pool(name="pmlp", bufs=1, space="PSUM") as pmlp,
        ):
            # --- constants (tiles now; DMAs after the first attr DMA so
            # the edge stream starts immediately) ---
            cs = cpool.tile([P, 5], f32, tag="consts")
            ws = cpool.tile([P, 4 * HIDDEN + 2 * DOUT], f16, tag="wts")
            c16 = cpool.tile([P, CBmax * W + NCH], f16, tag="c16")
            w1s_0 = ws[:, 0:HIDDEN]
            w1s_1 = ws[:, HIDDEN : 2 * HIDDEN]
            w2s_0 = ws[:, 2 * HIDDEN : 3 * HIDDEN]
            w2s_1 = ws[:, 3 * HIDDEN : 4 * HIDDEN]
            w3s_0 = ws[:, 4 * HIDDEN : 4 * HIDDEN + DOUT]
            w3s_1 = ws[:, 4 * HIDDEN + DOUT : 4 * HIDDEN + 2 * DOUT]
            b1s_0 = cs[:, 0:1]
            b1s_1 = cs[:, 1:2]
            b2s_0 = cs[:, 2:3]
            b2s_1 = cs[:, 3:4]
            b3s = cs[:, 4:5]
            it16 = c16[:, 0 : CBmax * W]
            dstrel_s = c16[:, CBmax * W : CBmax * W + NCH]
            oall = cpool.tile([P, NPC], f16, tag="oall")

            # group sizes: GPW windows each, tapering at the tail to
            # shorten the pipeline drain (last windows are also the
            # smallest thanks to the descending-count permutation)
            gsizes = _group_sizes()
            gstart = [0]
            for s in gsizes:
                gstart.append(gstart[-1] + s)

            for j in range(WINDOWS):
                off, cb, fullc, tmode = wmeta[j]
                g = next(i for i in range(len(gsizes)) if gstart[i + 1] > j)
                sw = j - gstart[g]
                gsz = gsizes[g]

                if sw == 0:
                    # one combined edge-feature + node-feature DMA per group
                    goff = off
                    jl = gstart[g + 1] - 1
                    gend = int(wmeta[jl][0] + wmeta[jl][1])
                    gw = (gend - goff) * D + gsz * W
                    gsrc = goff * D + gstart[g] * W
                    at = apool.tile([P, GPW * (CBmax * D + W)], f16, tag="attr")
                    nc.sync.dma_start(
                        out=at[:, :gw], in_=attrT_d[:, gsrc : gsrc + gw]
                    )
                    if j == 0:
                        nc.sync.dma_start(out=c16[:], in_=c16_d[:, :])
                        nc.sync.dma_start(out=cs[:], in_=consts_d[:, :])
                        nc.sync.dma_start(out=ws[:], in_=wts_d[:, :])
                    # flush the previous group's finished output slice
                    if g > 0 and gsizes[g - 1] == GPW:
                        f0, f1 = gstart[g - 1] * W, gstart[g] * W
                        nc.sync.dma_start(
                            out=outT_d[:, f0:f1], in_=oall[:, f0:f1]
                        )
                woff = off - goff  # window's chunk offset within group tile

                oh = ohpool.tile([P, CBmax * W], f16, tag="oh")
                if "oh" not in ablate:
                    nc.vector.tensor_tensor(
                        out=oh[:, : cb * W].rearrange("p (c m) -> p c m", m=W),
                        in0=dstrel_s[:, off : off + cb].to_broadcast([P, cb, W]),
                        in1=it16[:, : cb * W].rearrange("p (c m) -> p c m", m=W),
                        op=mybir.AluOpType.is_equal,
                    )

                pm = pbin.tile([P, W], f32, tag="mean")
                if "bin" not in ablate:
                    # (chunk-index-in-window, partition range) per matmul
                    if tmode == 1:      # shared half chunk last, rows 0:64
                        parts = [(ch, 0, P) for ch in range(fullc)]
                        parts.append((fullc, 0, 64))
                    elif tmode == 2:    # shared half chunk first, rows 64:128
                        parts = [(0, 64, P)]
                        parts += [(ch, 0, P) for ch in range(1, cb)]
                    else:
                        parts = [(ch, 0, P) for ch in range(cb)]
                    for i, (ch, p0, p1) in enumerate(parts):
                        nc.tensor.matmul(
                            out=pm[:],
                            lhsT=at[p0:p1, (woff + ch) * D : (woff + ch + 1) * D],
                            rhs=oh[p0:p1, ch * W : (ch + 1) * W],
                            start=(i == 0),
                            stop=(i == len(parts) - 1),
                        )

                if sw == 0:
                    mean_g = actpool.tile([P, GROUP_N], f16, tag="mean_g")
                if "bin" not in ablate:
                    nc.scalar.copy(out=mean_g[:, sw * W : (sw + 1) * W], in_=pm[:])

                if ("mlp" not in ablate) and (sw == gsz - 1):
                    # --- MLP over this group of nodes (feature-major) ---
                    NW = gsz * W
                    n0 = gstart[g] * W

                    ph1a = pmlp.tile([P, GROUP_N], f32, tag="h1a")
                    ph1b = pmlp.tile([P, GROUP_N], f32, tag="h1b")
                    nc.tensor.matmul(out=ph1a[:, :NW], lhsT=w1s_0[:, 0:P],
                                     rhs=at[:, (gend - goff) * D : (gend - goff) * D + NW], start=True, stop=False)
                    nc.tensor.matmul(out=ph1a[:, :NW], lhsT=w1s_1[:, 0:P],
                                     rhs=mean_g[:, :NW], start=False, stop=True)
                    nc.tensor.matmul(out=ph1b[:, :NW], lhsT=w1s_0[:, P:HIDDEN],
                                     rhs=at[:, (gend - goff) * D : (gend - goff) * D + NW], start=True, stop=False)
                    nc.tensor.matmul(out=ph1b[:, :NW], lhsT=w1s_1[:, P:HIDDEN],
                                     rhs=mean_g[:, :NW], start=False, stop=True)
                    h1a = actpool.tile([P, GROUP_N], f16, tag="h1a_s")
                    h1b = actpool.tile([P, GROUP_N], f16, tag="h1b_s")
                    nc.scalar.activation(out=h1a[:, :NW], in_=ph1a[:, :NW],
                                         func=Relu, bias=b1s_0[:, 0:1])
                    nc.scalar.activation(out=h1b[:, :NW], in_=ph1b[:, :NW],
                                         func=Relu, bias=b1s_1[:, 0:1])

                    ph2a = pmlp.tile([P, GROUP_N], f32, tag="h2a")
                    ph2b = pmlp.tile([P, GROUP_N], f32, tag="h2b")
                    nc.tensor.matmul(out=ph2a[:, :NW], lhsT=w2s_0[:, 0:P],
                                     rhs=h1a[:, :NW], start=True, stop=False)
                    nc.tensor.matmul(out=ph2a[:, :NW], lhsT=w2s_1[:, 0:P],
                                     rhs=h1b[:, :NW], start=False, stop=True)
                    nc.tensor.matmul(out=ph2b[:, :NW], lhsT=w2s_0[:, P:HIDDEN],
                                     rhs=h1a[:, :NW], start=True, stop=False)
                    nc.tensor.matmul(out=ph2b[:, :NW], lhsT=w2s_1[:, P:HIDDEN],
                                     rhs=h1b[:, :NW], start=False, stop=True)
                    h2a = actpool.tile([P, GROUP_N], f16, tag="h2a_s")
                    h2b = actpool.tile([P, GROUP_N], f16, tag="h2b_s")
                    nc.scalar.activation(out=h2a[:, :NW], in_=ph2a[:, :NW],
                                         func=Relu, bias=b2s_0[:, 0:1])
                    nc.scalar.activation(out=h2b[:, :NW], in_=ph2b[:, :NW],
                                         func=Relu, bias=b2s_1[:, 0:1])

                    po = pmlp.tile([P, GROUP_N], f32, tag="o")
                    nc.tensor.matmul(out=po[:, :NW], lhsT=w3s_0[:],
                                     rhs=h2a[:, :NW], start=True, stop=False)
                    nc.tensor.matmul(out=po[:, :NW], lhsT=w3s_1[:],
                                     rhs=h2b[:, :NW], start=False, stop=True)
                    nc.scalar.activation(out=oall[:, n0 : n0 + NW],
                                         in_=po[:, :NW],
                                         func=Ident, bias=b3s[:, 0:1])
                    if gsz < GPW:
                        # tail taper groups: no more prefetches to protect,
                        # store immediately to shorten the drain
                        nc.sync.dma_start(
                            out=outT_d[:, n0 : n0 + NW], in_=oall[:, n0 : n0 + NW]
                        )

            if gsizes[-1] == GPW:
                f0 = gstart[len(gsizes) - 1] * W
                nc.sync.dma_start(out=outT_d[:, f0:], in_=oall[:, f0:])

    # run_bass_via_pjrt (axon path) does not finalize; Bacc needs
    # finalize() to run its compile passes (reg alloc, wait legalization).
    nc.finalize()
    return nc


def _host_prep(x, edge_index, edge_attr):
    """Sort/scale/pad edges; returns (CB, per-core input arrays)."""
    col = np.asarray(edge_index)[1].astype(np.int64)
    x = np.asarray(x, dtype=np.float32)
    counts = np.bincount(col, minlength=N_NODES)
    scale = (1.0 / np.maximum(counts, 1)).astype(np.float32)

    order = np.argsort(col, kind="stable")
    col_s = col[order]
    attr_s = np.asarray(edge_attr, dtype=np.float32)[order]
    attr_s = attr_s * scale[col_s][:, None]

    # per-core, per-window edge counts
    starts = np.empty((N_CORES, WINDOWS + 1), dtype=np.int64)
    for c in range(N_CORES):
        bounds = np.minimum(
            c * NPC_REAL + np.arange(WINDOWS + 1) * W, (c + 1) * NPC_REAL
        )
        starts[c] = np.searchsorted(col_s, bounds)
    cnt = np.diff(starts, axis=1)  # [N_CORES, WINDOWS]

    # Each core processes its windows sorted by descending edge count.
    # Window slot j then holds every core's j-th order statistic, so the
    # cross-core max (the chunk plan must be shared, the program is SPMD)
    # wastes far less padding than positional assignment.  Small windows
    # land last, which also shortens the pipeline drain.  Host un-permutes
    # outputs.
    order = np.argsort(-cnt, axis=1, kind="stable")  # [N_CORES, WINDOWS]
    cnt_s = np.take_along_axis(cnt, order, axis=1)

    # Shared tail chunks: windows whose (cross-core max) remainder fits in
    # 64 edges can pair up, two tails sharing one 128-row chunk (A in
    # partitions 0:64, B in 64:128).  Reorder slots inside each group so
    # tailable windows are adjacent; odd leftovers get promoted to a full
    # chunk.
    m = cnt_s.max(axis=0)
    fullc = (m // P).astype(np.int64)
    rem = m - fullc * P
    fullc += rem > 64                     # big remainders stay full chunks
    tailable = ((rem > 0) & (rem <= 64)) | (m == 0)

    gsz_list = _group_sizes()
    slot_perm = []
    tmode = np.zeros(WINDOWS, np.int64)   # 0 none, 1 A(rows 0:64), 2 B(64:128)
    pos = 0
    for gs in gsz_list:
        idx = np.arange(pos, pos + gs)
        tl = idx[tailable[idx]]
        nont = idx[~tailable[idx]]
        if len(tl) % 2 == 1:              # promote one leftover tail
            lone = tl[-1]
            tl = tl[:-1]
            fullc[lone] += (rem[lone] > 0) | (m[lone] == 0)
            nont = np.append(nont, lone)
        slot_perm.extend(nont.tolist())
        slot_perm.extend(tl.tolist())
        tmode[pos + len(nont) : pos + gs] = np.tile([1, 2], len(tl) // 2)
        pos += gs
    slot_perm = np.asarray(slot_perm)
    fullc = fullc[slot_perm]
    order = order[:, slot_perm]
    cnt_s = cnt_s[:, slot_perm]

    # column offsets: A's shared column is also B's first column
    col_off = np.zeros(WINDOWS, np.int64)
    ncols = np.zeros(WINDOWS, np.int64)
    co = 0
    for j in range(WINDOWS):
        if tmode[j] == 2:
            col_off[j] = co - 1
            ncols[j] = fullc[j] + 1
            co += fullc[j]
        elif tmode[j] == 1:
            col_off[j] = co
            ncols[j] = fullc[j] + 1
            co += fullc[j] + 1
        else:
            col_off[j] = co
            ncols[j] = fullc[j]
            co += fullc[j]
    NCH = int(co)
    E_pad = NCH * P
    wmeta = tuple(
        (int(col_off[j]), int(ncols[j]), int(fullc[j]), int(tmode[j]))
        for j in range(WINDOWS)
    )

    per_core = []
    for c in range(N_CORES):
        ordc = order[c]
        cnts = cnt_s[c]                      # counts in processing order
        total = int(cnts.sum())
        # edge source rows (into col_s/attr_s), in processing order
        src_idx = np.concatenate(
            [np.arange(starts[c, w], starts[c, w + 1]) for w in ordc]
        )
        within = np.arange(total) - np.repeat(np.cumsum(cnts) - cnts, cnts)
        co_e = np.repeat(col_off, cnts)
        fc_e = np.repeat(fullc, cnts)
        tm_e = np.repeat(tmode, cnts)
        # rows: mode 0/1 fill columns contiguously (tail rows start at row
        # 0 of the last column); mode B fills its full columns (one past
        # the shared one) first, remainder into rows 64: of the shared.
        edest = co_e * P + within
        isB = tm_e == 2
        infull = within < fc_e * P
        edest[isB & infull] = (co_e * P + P + within)[isB & infull]
        edest[isB & ~infull] = (co_e * P + 64 + (within - fc_e * P))[
            isB & ~infull
        ]

        attr_pad = np.zeros((E_pad, D), np.float32)
        attr_pad[edest] = attr_s[src_idx]
        attrT_edges = (
            attr_pad.reshape(NCH, P, D)
            .transpose(1, 0, 2)
            .reshape(P, NCH * D)
            .astype(np.float16)
        )

        # dst relative to the processed window's node base
        win_base_proc = c * NPC_REAL + ordc * W  # global node base per slot
        dstrel = np.full((E_pad,), 200.0, np.float16)
        dstrel[edest] = (
            col_s[src_idx] - np.repeat(win_base_proc, cnts)
        ).astype(np.float16)
        dstrelT = np.ascontiguousarray(dstrel.reshape(NCH, P).T)

        # node features per 64-node window slot, zero-padded per slot
        xc = np.zeros((WINDOWS, W, D), np.float16)
        for j, w in enumerate(ordc):
            n0 = c * NPC_REAL + w * W
            n1 = min(n0 + W, (c + 1) * NPC_REAL)
            xc[j, : n1 - n0] = x[n0:n1].astype(np.float16)
        xT = xc.reshape(NPC, D).T  # [D, NPC]

        # interleave per group: [edge chunks | node features]
        gsizes = _group_sizes()
        attrT = np.empty((P, NCH * D + WINDOWS * W), np.float16)
        pos = 0
        j0 = 0
        for gsz in gsizes:
            c0 = int(col_off[j0])
            c1 = int(col_off[j0 + gsz - 1] + ncols[j0 + gsz - 1])
            wgt = (c1 - c0) * D
            attrT[:, pos : pos + wgt] = attrT_edges[:, c0 * D : c1 * D]
            pos += wgt
            attrT[:, pos : pos + gsz * W] = xT[:, j0 * W : (j0 + gsz) * W]
            pos += gsz * W
            j0 += gsz
        assert pos == attrT.shape[1] and j0 == WINDOWS

        per_core.append(
            {"attrT": np.ascontiguousarray(attrT), "dstrelT": dstrelT,
             "order": ordc}
        )
    return (NCH, wmeta), per_core


def _build_consts(b1, b2, b3):
    consts = np.zeros((P, 5), np.float32)
    consts[:, 0] = b1[:P]
    consts[:, 1] = b1[P:]
    consts[:, 2] = b2[:P]
    consts[:, 3] = b2[P:]
    consts[:, 4] = b3
    return consts


def _build_wts(W1, W2, W3):
    wts = np.empty((P, 4 * HIDDEN + 2 * DOUT), np.float16)
    wts[:, 0:HIDDEN] = W1[:P]
    wts[:, HIDDEN : 2 * HIDDEN] = W1[P:]
    wts[:, 2 * HIDDEN : 3 * HIDDEN] = W2[:P]
    wts[:, 3 * HIDDEN : 4 * HIDDEN] = W2[P:]
    wts[:, 4 * HIDDEN : 4 * HIDDEN + DOUT] = W3[:P]
    wts[:, 4 * HIDDEN + DOUT : 4 * HIDDEN + 2 * DOUT] = W3[P:]
    return wts


def _build_c16(META, dstrelT):
    """fp16 consts row-block: iota ramp | dstrel."""
    NCH, wmeta = META
    CBmax = max(mw[1] for mw in wmeta)
    c16 = np.empty((P, CBmax * W + NCH), np.float16)
    c16[:, 0 : CBmax * W] = np.tile(np.arange(W, dtype=np.float16), CBmax)[None, :]
    c16[:, CBmax * W :] = dstrelT
    return c16


def kernel(x, edge_index, edge_attr, W1, b1, W2, b2, W3, b3):
    CB, per_core = _host_prep(x, edge_index, edge_attr)

    key = CB
    if key not in _prog_cache:
        _prog_cache[key] = _build_program(CB)
    nc = _prog_cache[key]

    W1 = np.asarray(W1, np.float32)
    W2 = np.asarray(W2, np.float32)
    W3 = np.asarray(W3, np.float32)
    b1 = np.asarray(b1, np.float32)
    b2 = np.asarray(b2, np.float32)
    b3 = np.asarray(b3, np.float32)
    consts = _build_consts(b1, b2, b3)
    wts = _build_wts(W1, W2, W3)
    in_maps = [
        {
            "attrT": pc["attrT"],
            "c16": _build_c16(CB, pc["dstrelT"]),
            "consts": consts,
            "wts": wts,
        }
        for pc in per_core
    ]

    res = run_bass_kernel_spmd(nc, in_maps, core_ids=list(range(N_CORES)))

    out = np.empty((N_NODES, DOUT), np.float32)
    for c in range(N_CORES):
        o = res.results[c]["outT"].T.astype(np.float32).reshape(WINDOWS, W, DOUT)
        for j, w in enumerate(per_core[c]["order"]):
            n0 = c * NPC_REAL + int(w) * W
            n1 = min(n0 + W, (c + 1) * NPC_REAL)
            out[n0:n1] = o[j, : n1 - n0]
    return out
